# revision 23
# baseline (speedup 1.0000x reference)
"""Trainium2 Bass kernel: GAT-style attention layer, data-parallel over 8 NeuronCores.

Reference computation (per node n, K=32 neighbors, D=128 features, L=64 labels):
    h     = lrelu(x @ W)                  [N,K,D]
    e     = lrelu(h @ v + bias)           [N,K,1]
    alpha = softmax_k(e)                  [N,K]
    out   = sum_k alpha[n,k] * labels[n,k,:]   [N,L]

Sharding: pure data parallel over nodes (6250/core, zero-padded to 6400).
Host side re-lays each core's shard out so every DMA is contiguous per
partition; values are untouched (fp32 in DRAM).

Device pipeline per 256-node tile (software-pipelined by one tile so the
TensorE never idles and its HAM clock stays at 2.4 GHz):
  mm1   z^T[e,(k,n)] = W^T @ x^T          TensorE bf16, PSUM f32
  lrelu PSUM->SBUF bf16 (ScalarE Prelu, fused activation)
  mm2   s[k,n] = v^T @ h^T via selector weights, PSUM-accumulated rows
  e=lrelu(s+bias), w=exp(e)               ScalarE (bias is per-partition AP)
  w^T   TensorE transpose -> [n,k]; row sums via ScalarE accum_out
  alpha = w^T * (1/sums)                  VectorE (per-partition scalar)
  diag(alpha_k) tiles on VectorE (identity mask * scalar)
  agg   out^T = sum_k labels_k^T-as-stationary @ diag(alpha_k)
        PSUM-accumulated; interleaved into the NEXT tile's mm1 stream
"""
import sys

sys.path.insert(0, "/opt/trn_rl_repo")
import numpy as np

N, K, D, L = 50000, 32, 128, 64
NEG = 0.2
NCORES = 8
NPER = N // NCORES          # 6250
TN = 256                    # nodes per tile
NSUB = TN // 128            # sub-tiles of 128 nodes
NPAD = 6400                 # padded nodes per core
NT = NPAD // TN             # 25 tiles

LAST_RESULT = None
_cache = {}


def build(nt):
    import concourse.bass as bass
    import concourse.tile as tile
    from concourse import bacc, mybir

    f32 = mybir.dt.float32
    bf16 = mybir.dt.bfloat16
    AF = mybir.ActivationFunctionType
    OP = mybir.AluOpType
    PSUM = bass.MemorySpace.PSUM

    nc = bacc.Bacc(
        "TRN2", target_bir_lowering=False, debug=False, num_devices=NCORES
    )
    x_ext = nc.declare_dram_parameter("x", [nt, 128, K * TN], f32, False)
    lab_ext = nc.declare_dram_parameter("lab", [nt, 128, NSUB * K * L], f32, False)
    w_ext = nc.declare_dram_parameter("w", [D, D], f32, False)
    v_ext = nc.declare_dram_parameter("v", [D, 1], f32, False)
    b_ext = nc.declare_dram_parameter("b", [K, 1], f32, False)
    # out^T per tile: [L, NSUB*128] (transposed back on the host)
    out_ext = nc.declare_dram_parameter("out", [nt, L, NSUB * 128], f32, isOutput=True)

    with tile.TileContext(nc) as tc:
        with (
            tc.tile_pool(name="const", bufs=1) as const,
            tc.tile_pool(name="xp", bufs=3) as xp,
            tc.tile_pool(name="labp", bufs=3) as labp,
            tc.tile_pool(name="hp", bufs=2) as hp,
            tc.tile_pool(name="wp", bufs=2) as wp,
            tc.tile_pool(name="smallp", bufs=4) as smallp,
            tc.tile_pool(name="dkp", bufs=2) as dkp,
            tc.tile_pool(name="outp", bufs=2) as outp,
            tc.tile_pool(name="zps", bufs=3, space=PSUM) as zps,
            tc.tile_pool(name="sps", bufs=2, space=PSUM) as sps,
            tc.tile_pool(name="wtps", bufs=1, space=PSUM) as wtps,
            tc.tile_pool(name="aps", bufs=1, space=PSUM) as aps,
        ):
            W_sb = const.tile([128, 128], bf16)
            nc.gpsimd.dma_start(W_sb[:], w_ext[:])      # f32 -> bf16 cast DMA
            v_sb = const.tile([128, 1], bf16)
            nc.gpsimd.dma_start(v_sb[:], v_ext[:])
            bias_sb = const.tile([32, 1], f32)
            nc.sync.dma_start(bias_sb[:], b_ext[:])
            ones = const.tile([128, 128], bf16)
            nc.vector.memset(ones[:], 1.0)
            mask = const.tile([128, 128], bf16)         # identity matrix
            nc.gpsimd.affine_select(
                mask[:], ones[:], pattern=[[1, 128]],
                compare_op=OP.is_equal, fill=0.0, base=0, channel_multiplier=-1,
            )
            # vks[:, 32k+m] = v * (m == k): selector weights so score matmul k
            # writes only PSUM row k of a [32, TN] tile (base partition stays 0)
            vks = const.tile([128, K * 32], bf16)
            nc.vector.memset(vks[:], 0.0)
            nc.vector.tensor_copy(
                vks[:, 0:K * 32:33], v_sb[:, 0:1].broadcast_to([128, 32])
            )

            nchunk = (K * TN) // 512     # 16 mm1 chunks per tile
            prev = None                  # state of tile t-1 awaiting aggregation

            def emit_softmax_tail(st):
                """TensorE transpose of exp-weights + normalized diag(alpha)
                tiles for tile `st['t']`; runs while that tile's aggregation
                is interleaved into the next tile's mm1 stream."""
                w_sb = st["w_sb"]
                dk_all = dkp.tile([128, K * 2 * 128], bf16, tag="dk")
                for s in range(NSUB):
                    wT_ps = wtps.tile([128, 32], bf16)
                    nc.tensor.transpose(
                        wT_ps[:], w_sb[:, s * 128:(s + 1) * 128], mask[0:32, 0:32]
                    )
                    wT_sb = smallp.tile([128, 32], f32)
                    sums = smallp.tile([128, 1], f32)
                    nc.scalar.activation(wT_sb[:], wT_ps[:], AF.Copy, accum_out=sums[:])
                    recip = smallp.tile([128, 1], f32)
                    nc.vector.reciprocal(recip[:], sums[:])
                    alphaN = smallp.tile([128, 32], f32)
                    nc.vector.tensor_scalar_mul(alphaN[:], wT_sb[:], recip[:, 0:1])
                    for k in range(K):
                        nc.vector.tensor_scalar_mul(
                            dk_all[:, (s * K + k) * 128:(s * K + k + 1) * 128],
                            mask[:], alphaN[:, k:k + 1],
                        )
                st["dk_all"] = dk_all
                # two PSUM accumulation groups (one per 128-node sub-tile)
                st["a_ps"] = [
                    aps.tile([L, 128], f32, name=f"a_ps{s}", tag=f"agg{s}")
                    for s in range(NSUB)
                ]
                st["agg"] = [
                    (s, k, st["a_ps"][s]) for s in range(NSUB) for k in range(K)
                ]

            def emit_agg_matmuls(st, items):
                for s, k, a_ps in items:
                    nc.tensor.matmul(
                        a_ps[:],
                        st["lab_sb"][:, (s * K + k) * L:(s * K + k + 1) * L],
                        st["dk_all"][:, (s * K + k) * 128:(s * K + k + 1) * 128],
                        start=(k == 0), stop=(k == K - 1),
                    )

            def emit_agg_finish(st):
                out_sb = outp.tile([L, NSUB * 128], f32)
                for s in range(NSUB):
                    nc.scalar.activation(
                        out_sb[:, s * 128:(s + 1) * 128], st["a_ps"][s][:], AF.Copy
                    )
                nc.sync.dma_start(out_ext[st["t"]], out_sb[:])

            for t in range(nt):
                x_sb = xp.tile([128, K * TN], bf16)
                q4 = K * TN // 4
                for qi in range(4):
                    nc.gpsimd.dma_start(
                        x_sb[:, qi * q4:(qi + 1) * q4], x_ext[t][:, qi * q4:(qi + 1) * q4]
                    )
                lab_sb = labp.tile([128, NSUB * K * L], bf16)
                lhalf = NSUB * K * L // 2
                nc.gpsimd.dma_start(lab_sb[:, 0:lhalf], lab_ext[t][:, 0:lhalf])
                nc.gpsimd.dma_start(lab_sb[:, lhalf:], lab_ext[t][:, lhalf:])

                h_sb = hp.tile([128, K * TN], bf16)
                s_ps = sps.tile([32, TN], f32)

                def emit_mm2(k):
                    nc.tensor.matmul(
                        s_ps[:], vks[:, k * 32:(k + 1) * 32],
                        h_sb[:, k * TN:(k + 1) * TN],
                        start=(k == 0), stop=(k == K - 1),
                    )

                # chunk 0, then the previous tile's softmax tail (transpose on
                # PE + diag builds on DVE) so its agg matmuls are ready to
                # interleave into the remaining chunks. mm2 for chunk c runs
                # one chunk late so the ScalarE lrelu latency is hidden.
                z_ps = zps.tile([128, 512], f32)
                nc.tensor.matmul(z_ps[:], W_sb[:], x_sb[:, 0:512])
                nc.scalar.activation(h_sb[:, 0:512], z_ps[:], AF.Prelu, alpha=NEG)
                if prev is not None:
                    emit_softmax_tail(prev)
                    agg_items = list(prev["agg"])
                for c in range(1, nchunk):
                    z_ps = zps.tile([128, 512], f32)
                    nc.tensor.matmul(z_ps[:], W_sb[:], x_sb[:, c * 512:(c + 1) * 512])
                    nc.scalar.activation(
                        h_sb[:, c * 512:(c + 1) * 512], z_ps[:], AF.Prelu, alpha=NEG
                    )
                    if prev is not None:
                        take, agg_items = agg_items[:4], agg_items[4:]
                        emit_agg_matmuls(prev, take)
                    emit_mm2(2 * (c - 1))
                    emit_mm2(2 * (c - 1) + 1)
                if prev is not None:
                    emit_agg_matmuls(prev, agg_items)  # remaining 4
                    emit_agg_finish(prev)
                emit_mm2(2 * nchunk - 2)
                emit_mm2(2 * nchunk - 1)

                e_sb = wp.tile([32, TN], f32)
                nc.scalar.activation(
                    e_sb[:], s_ps[:], AF.Prelu, bias=bias_sb[:, 0:1], alpha=NEG
                )
                w_sb = wp.tile([32, TN], bf16)
                nc.scalar.activation(w_sb[:], e_sb[:], AF.Exp)

                prev = {"t": t, "w_sb": w_sb, "lab_sb": lab_sb}

            # drain the last tile
            emit_softmax_tail(prev)
            emit_agg_matmuls(prev, prev["agg"])
            emit_agg_finish(prev)
    nc.compile()
    return nc


def shard_inputs(x, lab, nt=NT, nper=NPER):
    npad = nt * TN
    xs = np.zeros((npad, K, D), np.float32)
    xs[:nper] = x
    ls = np.zeros((npad, K, L), np.float32)
    ls[:nper] = lab
    xf = np.ascontiguousarray(
        xs.reshape(nt, TN, K, D).transpose(0, 3, 2, 1)
    ).reshape(nt, 128, K * TN)
    lf = np.ascontiguousarray(
        ls.reshape(nt, NSUB, 128, K * L).transpose(0, 2, 1, 3)
    ).reshape(nt, 128, NSUB * K * L)
    return xf, lf


def unshard_output(o, nt=NT, nper=NPER):
    # o is [nt, L, NSUB*128] (transposed on device); node = t*TN + col
    return np.ascontiguousarray(o.transpose(0, 2, 1)).reshape(nt * TN, L)[:nper]


def kernel(para_neighbors, para_nei_labels, linear, e_vec, bias):
    from concourse.bass_utils import run_bass_kernel_spmd

    global LAST_RESULT
    x = np.asarray(para_neighbors, np.float32)
    lab = np.asarray(para_nei_labels, np.float32)
    Wm = np.ascontiguousarray(np.asarray(linear, np.float32))
    v = np.ascontiguousarray(np.asarray(e_vec, np.float32))
    b = np.ascontiguousarray(np.asarray(bias, np.float32))

    if "nc" not in _cache:
        _cache["nc"] = build(NT)
    nc = _cache["nc"]

    in_maps = []
    for i in range(NCORES):
        xf, lf = shard_inputs(x[i * NPER:(i + 1) * NPER], lab[i * NPER:(i + 1) * NPER])
        in_maps.append({"x": xf, "lab": lf, "w": Wm, "v": v, "b": b})

    res = run_bass_kernel_spmd(nc, in_maps, core_ids=list(range(NCORES)))
    LAST_RESULT = res
    outs = [unshard_output(res.results[i]["out"]) for i in range(NCORES)]
    return np.ascontiguousarray(np.concatenate(outs, axis=0))
